# revision 16
# baseline (speedup 1.0000x reference)
"""Trainium2 Bass kernel for nn_DataEmbedding_v2 (circular conv1d + sinusoidal PE
+ causal threshold-scan "tc" embedding).

Contract: kernel(**inputs) takes FULL inputs (x:(16,2048,64) f32, conv_w:(512,64,3),
tc_w:(512,1), tc_b:(512,)) and returns the FULL (16,2048,512) f32 output.
Internally: batch-sharded data-parallel across 8 NeuronCores (2 batches/core),
params replicated.

v2 design notes:
- tc plane runs in [t-partitions, s-free] layout: the compare is a
  tensor_scalar (threshold = per-partition scalar, norms = broadcast row) and
  the masked max over s is a fused tensor_tensor_reduce with a (s+1) row
  const (history) / causal-masked global-index const (diagonal). No
  cross-partition reduce, no transposes at the end.
- Compares are exact fp32 (min |norms[s]-th[t]| on this input is 1.1e-5, so
  fp16 anywhere in the compare path would flip taus).
- x is transposed/tap-shifted on the host into fp16 lhsT layouts; conv per
  128-token tile is 3 back-to-back matmuls into one PSUM group: identity@pe,
  taps01 (K=128), tap2+tau (K=65, tau is one fp16 row: tau<=2048 exact, and
  tau*err(w16) <= ~2 abs vs the 24 abs error budget).
- Output staged as fp16 (ACT cast-copies from PSUM), DMAed per 4 tiles,
  upcast to f32 on the host.
"""

import math
import os
import sys

sys.path.insert(0, "/opt/trn_rl_repo")

import numpy as np

B, S, C, D = 16, 2048, 64, 512
NCORES = 8
BLOC = B // NCORES  # batches per core
P = 128
NT = S // P  # 16 tiles of 128 tokens
ETA = 0.3
EPS = 1e-08

# tensor_tensor_reduce faults on real HW (sim-only path); default to the
# tensor_tensor + tensor_reduce pair.
NO_TTR = bool(int(os.environ.get("V2_NO_TTR", "1")))


def _emit(tc, aps):
    """Emit the per-core Tile kernel. aps: dict of DRAM APs."""
    from contextlib import ExitStack

    from concourse import mybir

    f32 = mybir.dt.float32
    f16 = mybir.dt.float16
    Alu = mybir.AluOpType
    Ax = mybir.AxisListType

    nc = tc.nc
    xin, xt, x2t = aps["xin"], aps["xt"], aps["x2t"]
    pe16, wt01, wt2x, identh = aps["pe16"], aps["wt01"], aps["wt2x"], aps["identh"]
    seqr, dconst, t1ct = aps["seqr"], aps["dconst"], aps["t1ct"]
    out, nt_dram, tau_dram = aps["out"], aps["nt_dram"], aps["tau_dram"]

    with ExitStack() as ctx:
        singles = ctx.enter_context(tc.tile_pool(name="singles", bufs=1))
        xinp = ctx.enter_context(tc.tile_pool(name="xinp", bufs=2))
        xtp = ctx.enter_context(tc.tile_pool(name="xtp", bufs=2))
        x2p = ctx.enter_context(tc.tile_pool(name="x2p", bufs=2))
        nrowp = ctx.enter_context(tc.tile_pool(name="nrowp", bufs=2))
        osbp = ctx.enter_context(tc.tile_pool(name="osbp", bufs=2))
        small = ctx.enter_context(tc.tile_pool(name="small", bufs=2))
        tiny = ctx.enter_context(tc.tile_pool(name="tiny", bufs=2))
        predp = ctx.enter_context(tc.tile_pool(name="predp", bufs=2))
        scrp = ctx.enter_context(tc.tile_pool(name="scrp", bufs=2))
        psA = ctx.enter_context(tc.tile_pool(name="psA", bufs=4, space="PSUM"))
        psT = ctx.enter_context(tc.tile_pool(name="psT", bufs=2, space="PSUM"))

        # ---- input loads. xin b1 on the gpsimd ring (so it runs parallel
        # with xin b0 on the sync ring); x-transposed fp16 tensors follow on
        # the sync ring; consts on the scalar ring. ----
        xin1_sb = xinp.tile([P, NT, C], f32, tag="xin", name="xin1")
        nc.gpsimd.dma_start(xin1_sb, xin[1].rearrange("(j p) c -> p j c", p=P))
        xin0_sb = xinp.tile([P, NT, C], f32, tag="xin", name="xin0")
        nc.sync.dma_start(xin0_sb, xin[0].rearrange("(j p) c -> p j c", p=P))

        xts = {}
        x2s = {}
        for b in range(BLOC):
            xt_sb = xtp.tile([P, S], f16, tag="xt", name=f"xt{b}")
            nc.sync.dma_start(xt_sb, xt[b])
            x2_sb = x2p.tile([65, S], f16, tag="x2", name=f"x2{b}")
            nc.sync.dma_start(x2_sb[0:64, :], x2t[b])
            xts[b], x2s[b] = xt_sb, x2_sb

        identh_sb = singles.tile([P, P], f16)
        nc.scalar.dma_start(identh_sb, identh)
        wt01_sb = singles.tile([P, D], f16)
        nc.scalar.dma_start(wt01_sb, wt01)
        wt2x_sb = singles.tile([65, D], f16)
        nc.scalar.dma_start(wt2x_sb, wt2x)
        seq_sb = singles.tile([P, S], f16)
        nc.scalar.dma_start(seq_sb, seqr)
        dconst_sb = singles.tile([P, NT, P], f16)
        nc.scalar.dma_start(dconst_sb, dconst)
        t1ct_sb = singles.tile([NT, P], f32)
        nc.scalar.dma_start(t1ct_sb, t1ct)

        nc.tensor.ldweights(identh_sb)  # absorbs identh DMA before transposes

        # ---- per-batch state ----
        st = {}

        def emit_norms(b):
            """L1 norms (exact ref fp32 order), thresh, norms roundtrip to a
            [128, S] partition-broadcast row."""
            xin_sb = xin0_sb if b == 0 else xin1_sb
            r8 = small.tile([P, NT, 8], f32, tag="r8", name=f"r8_{b}")
            nc.vector.tensor_reduce(
                r8,
                xin_sb.rearrange("p j (a z) -> p j a z", z=8),
                axis=Ax.X,
                op=Alu.add,
                apply_absolute_value=True,
            )
            normc = small.tile([P, NT], f32, tag="normc", name=f"normc{b}")
            nc.vector.tensor_reduce(normc, r8, axis=Ax.X, op=Alu.add)
            thc = small.tile([P, NT], f32, tag="thc", name=f"thc{b}")
            nc.vector.tensor_scalar(
                thc, normc, float(EPS), float(1.0 - ETA), op0=Alu.add, op1=Alu.mult
            )
            # p-major contiguous DRAM write (addr = p*NT + j), broadcast-read
            # into [128, S]. The plane's free axis enumerates s in this
            # permuted order (u = p*NT + j <-> s = j*P + p); max-reduce is
            # order-invariant and seqpm/dconst consts match the permutation.
            nc.scalar.dma_start(nt_dram[b].rearrange("(p j) -> p j", j=NT), normc)
            nrow = nrowp.tile([P, S], f32, tag="nrow", name=f"nrow{b}")
            nc.gpsimd.dma_start(nrow, nt_dram[b].partition_broadcast(P))
            st[b] = (thc, nrow)

        def emit_plane(b):
            """tc plane in [t-part, s-free]: per tile a fused compare +
            masked-max-reduce; res[t] = s_max+1 (global) or 0."""
            thc, nrow = st[b]
            res = small.tile([P, NT], f16, tag="res", name=f"res{b}")
            resh = small.tile([P, NT], f32, tag="resh", name=f"resh{b}")
            # p-major permuted views: free = (p_src, j), s = j*P + p_src
            nrow_v = nrow.rearrange("q (p j) -> q p j", j=NT)
            seq_v = seq_sb.rearrange("q (p j) -> q p j", j=NT)
            for i in range(NT):
                pred = predp.tile([P, P, NT], f16, tag="pred", name=f"pred{b}_{i}")
                scr = scrp.tile([P, P, NT], f16, tag="scr", name=f"scr{b}_{i}")
                # compare over history blocks j<i plus the diagonal block j=i
                nc.vector.tensor_scalar(
                    pred[:, :, 0 : i + 1],
                    nrow_v[:, :, 0 : i + 1],
                    thc[:, i : i + 1],
                    None,
                    op0=Alu.is_lt,
                )
                if NO_TTR:
                    if i > 0:
                        nc.vector.tensor_tensor(
                            scr[:, :, 0 : i + 1],
                            pred[:, :, 0 : i + 1],
                            seq_v[:, :, 0 : i + 1],
                            op=Alu.mult,
                        )
                        nc.vector.tensor_tensor(
                            scr[:, :, i],
                            pred[:, :, i],
                            dconst_sb[:, i, :],
                            op=Alu.mult,
                        )
                        nc.vector.tensor_reduce(
                            res[:, i : i + 1],
                            scr[:, :, 0 : i + 1],
                            axis=Ax.XY,
                            op=Alu.max,
                        )
                    else:
                        nc.vector.tensor_tensor(
                            scr[:, :, i], pred[:, :, i], dconst_sb[:, i, :], op=Alu.mult
                        )
                        nc.vector.tensor_reduce(
                            res[:, i : i + 1], scr[:, :, i], axis=Ax.X, op=Alu.max
                        )
                    continue
                if i > 0:
                    nc.vector.tensor_tensor_reduce(
                        out=scr[:, :, 0:i],
                        in0=pred[:, :, 0:i],
                        in1=seq_v[:, :, 0:i],
                        scale=1.0,
                        scalar=0.0,
                        op0=Alu.mult,
                        op1=Alu.max,
                        accum_out=resh[:, i : i + 1],
                    )
                nc.vector.tensor_tensor_reduce(
                    out=scr[:, :, i],
                    in0=pred[:, :, i],
                    in1=dconst_sb[:, i, :],
                    scale=1.0,
                    scalar=resh[:, i : i + 1] if i > 0 else 0.0,
                    op0=Alu.mult,
                    op1=Alu.max,
                    accum_out=res[:, i : i + 1],
                )
            # tau = (res>0) ? t+1-res : 0, computed in transposed [NT, P]
            # layout so the tau DRAM write is contiguous t-major fp16
            rt_ps = psT.tile([NT, P], f16, tag="pst", name=f"rt{b}")
            nc.tensor.transpose(rt_ps, res, identh_sb)
            td = tiny.tile([NT, P], f32, tag="td", name=f"td{b}")
            nc.vector.tensor_tensor(td, t1ct_sb, rt_ps, op=Alu.subtract)
            m01 = tiny.tile([NT, P], f16, tag="m01", name=f"m01{b}")
            nc.vector.tensor_scalar(m01, rt_ps, 0.0, None, op0=Alu.is_gt)
            tau16 = tiny.tile([NT, P], f16, tag="tau16", name=f"tau16{b}")
            nc.vector.tensor_tensor(tau16, td, m01, op=Alu.mult)
            nc.scalar.dma_start(tau_dram[b].rearrange("(i p) -> i p", p=P), tau16)
            # broadcast tau into row 0 of the tap2 lhsT tile
            nc.gpsimd.dma_start(x2s[b][64:65, :], tau_dram[b].partition_broadcast(1))

        def emit_conv(b):
            """Per tile: pe + conv taps + tau*tc_w as 3 matmuls into one PSUM
            group; ACT cast-copies to fp16 staging; DMA out per 4 tiles."""
            xt_sb, x2_sb = xts[b], x2s[b]
            osb = osbp.tile([P, NT, D], f16, tag="osb", name=f"osb{b}")
            for i in range(NT):
                ps = psA.tile([P, D], f32, tag="psa", name=f"ps{b}_{i}")
                nc.tensor.matmul(
                    ps, lhsT=identh_sb, rhs=pe16_sb[:, i, :], start=True, stop=False
                )
                nc.tensor.matmul(
                    ps,
                    lhsT=xt_sb[:, i * P : (i + 1) * P],
                    rhs=wt01_sb,
                    start=False,
                    stop=False,
                )
                nc.tensor.matmul(
                    ps,
                    lhsT=x2_sb[:, i * P : (i + 1) * P],
                    rhs=wt2x_sb,
                    start=False,
                    stop=True,
                )
                nc.scalar.copy(osb[:, i, :], ps)
                if i % 4 == 3:
                    q = i // 4
                    nc.sync.dma_start(
                        out[b, q * 4 * P : (q + 1) * 4 * P, :].rearrange(
                            "(i p) d -> p i d", p=P
                        ),
                        osb[:, q * 4 : (q + 1) * 4, :],
                    )

        # ---- schedule: norms b0/b1 early (xin b1 lands in parallel), then
        # PE priming (pe16 loads late on the ACT ring, so priming must not
        # precede the norm transposes in the PE queue), plane b0, conv b0
        # while plane b1 runs on DVE. ----
        emit_norms(0)
        emit_norms(1)

        # pe16 late on the ACT ring (everything above it is needed earlier)
        pe16_sb = singles.tile([P, NT, D], f16)
        nc.scalar.dma_start(pe16_sb, pe16.rearrange("(i p) d -> p i d", p=P))

        # PE priming: absorb const-DMA waits (a PE matmul carries at most
        # ONE sync wait). PSUM outputs dumped via ScalarE so later bank
        # reuse waits on ACT.
        prime_pe = psA.tile([P, D], f32, tag="psa")
        nc.tensor.matmul(
            prime_pe, lhsT=identh_sb, rhs=pe16_sb[:, 0, :], start=True, stop=True
        )  # absorbs pe16
        nc.tensor.ldweights(xts[0][:, 0:P])  # absorbs xt0
        prime_a = psA.tile([P, D], f32, tag="psa")
        nc.tensor.matmul(
            prime_a, lhsT=xts[0][:, 0:P], rhs=wt01_sb, start=True, stop=True
        )  # absorbs wt01
        nc.tensor.ldweights(x2s[0][0:64, 0:P])  # absorbs x2t0
        prime_b = psA.tile([P, D], f32, tag="psa")
        nc.tensor.matmul(
            prime_b, lhsT=x2s[0][0:64, 0:P], rhs=wt2x_sb[0:64, :], start=True, stop=True
        )  # absorbs wt2x
        dumps = singles.tile([P, 3], f32)
        nc.scalar.copy(dumps[:, 0:1], prime_pe[:, 0:1])
        nc.scalar.copy(dumps[:, 1:2], prime_a[:, 0:1])
        nc.scalar.copy(dumps[:, 2:3], prime_b[:, 0:1])

        emit_plane(0)
        # absorb b1 x-tensor DMA waits while PE is idle (loads done by now)
        nc.tensor.ldweights(xts[1][:, 0:P])
        nc.tensor.ldweights(x2s[1][0:64, 0:P])
        emit_conv(0)
        emit_plane(1)
        emit_conv(1)


def build_bass():
    """Build the per-core Bass module (traced once, then bacc-compiled)."""
    import concourse.tile as tile
    from concourse import bacc, mybir

    f32 = mybir.dt.float32
    f16 = mybir.dt.float16

    nc = bacc.Bacc(
        "TRN2",
        target_bir_lowering=False,
        debug=False,
        enable_asserts=False,
        num_devices=NCORES,
    )
    aps = {}
    aps["xin"] = nc.dram_tensor("xin", (BLOC, S, C), f32, kind="ExternalInput").ap()
    aps["xt"] = nc.dram_tensor("xt", (BLOC, P, S), f16, kind="ExternalInput").ap()
    aps["x2t"] = nc.dram_tensor("x2t", (BLOC, 64, S), f16, kind="ExternalInput").ap()
    aps["pe16"] = nc.dram_tensor("pe16", (S, D), f16, kind="ExternalInput").ap()
    aps["wt01"] = nc.dram_tensor("wt01", (P, D), f16, kind="ExternalInput").ap()
    aps["wt2x"] = nc.dram_tensor("wt2x", (65, D), f16, kind="ExternalInput").ap()
    aps["identh"] = nc.dram_tensor("identh", (P, P), f16, kind="ExternalInput").ap()
    aps["seqr"] = nc.dram_tensor("seqr", (P, S), f16, kind="ExternalInput").ap()
    aps["dconst"] = nc.dram_tensor("dconst", (P, NT, P), f16, kind="ExternalInput").ap()
    aps["t1ct"] = nc.dram_tensor("t1ct", (NT, P), f32, kind="ExternalInput").ap()
    aps["out"] = nc.dram_tensor("out", (BLOC, S, D), f16, kind="ExternalOutput").ap()
    aps["nt_dram"] = nc.dram_tensor("nt_scratch", (BLOC, S), f32, kind="Internal").ap()
    aps["tau_dram"] = nc.dram_tensor(
        "tau_scratch", (BLOC, S), f16, kind="Internal"
    ).ap()

    with tile.TileContext(nc) as tc:
        _emit(tc, aps)
    nc.compile()
    return nc


def make_consts():
    """Host-side constant tensors (replicated params + index helpers)."""
    # positional embedding, matching the reference formula in fp32
    position = np.arange(S, dtype=np.float32)[:, None]
    div_term = np.exp(
        np.arange(0, D, 2, dtype=np.float32) * np.float32(-math.log(10000.0) / D)
    ).astype(np.float32)
    ang = (position * div_term).astype(np.float32)
    pe = np.zeros((S, D), dtype=np.float32)
    pe[:, 0::2] = np.sin(ang)
    pe[:, 1::2] = np.cos(ang)

    # seq const in p-major permuted order: free index u = p*NT + j <-> s = j*P + p
    pg, jg = np.meshgrid(np.arange(P), np.arange(NT), indexing="ij")  # [p, j]
    s_of_u = (jg * P + pg).reshape(-1).astype(np.float32)  # [S] in u-order
    seqr = np.broadcast_to(s_of_u + 1.0, (P, S)).astype(np.float16)
    # dconst[p, i, c] = (i*128 + c + 1) if c < p else 0  (diag masked global s+1)
    pp = np.arange(P)
    cc = np.arange(P)
    caus = (cc[None, :] < pp[:, None]).astype(np.float32)  # [p, c]
    gidx = (np.arange(NT)[:, None] * P + cc[None, :] + 1.0).astype(np.float32)  # [i, c]
    dconst = (caus[:, None, :] * gidx[None, :, :]).astype(np.float16)  # [p, i, c]
    # t1ct[i, p] = i*128 + p + 1  (t+1 in transposed layout)
    t1ct = (np.arange(NT)[:, None] * P + np.arange(P)[None, :] + 1.0).astype(
        np.float32
    )
    consts = {
        "identh": np.eye(P, dtype=np.float16),
        "seqr": np.ascontiguousarray(seqr),
        "dconst": np.ascontiguousarray(dconst),
        "t1ct": np.ascontiguousarray(t1ct),
    }
    return pe, consts


def make_shared_inputs(conv_w, tc_w, tc_b):
    pe, consts = make_consts()
    pe_b = (pe + np.asarray(tc_b, np.float32)[None, :]).astype(np.float32)
    # conv weights, channel-major per tap: wk[c, d] = conv_w[d, c, k]
    wt = np.transpose(np.asarray(conv_w, np.float32), (2, 1, 0))  # (k, c, d)
    wt01 = np.concatenate([wt[0], wt[1]], axis=0).astype(np.float16)  # (128, D)
    # wt2x: row 0 = tc_w, rows 1:65 = tap2 weights
    w = np.asarray(tc_w, np.float32)[:, 0]
    wt2x = np.concatenate([wt[2], w[None, :]], axis=0).astype(np.float16)  # (65, D)
    return {
        "pe16": pe_b.astype(np.float16),
        "wt01": np.ascontiguousarray(wt01),
        "wt2x": np.ascontiguousarray(wt2x),
        **{k: np.ascontiguousarray(v) for k, v in consts.items()},
    }


_BUILD_CACHE = {}


def _install_ntff_hook():
    """The agent image's antenv lacks axon_hooks; synthesize it from the
    boot module's ctypes implementation so trace=True works under axon."""
    import sys as _sys
    import types

    if "antenv.axon_hooks" in _sys.modules:
        return
    try:
        from trn_agent_boot.trn_boot import _ntff_profile_via_ctypes

        hook = _ntff_profile_via_ctypes("/opt/axon/libaxon_pjrt.so")
        m = types.ModuleType("antenv.axon_hooks")
        m.get_axon_ntff_profile_hook = lambda: hook
        _sys.modules["antenv.axon_hooks"] = m
    except Exception as e:  # degrade to no-trace
        print("[kernel] ntff hook install failed:", e)


def kernel(x, conv_w, tc_w, tc_b):
    x = np.ascontiguousarray(np.asarray(x, dtype=np.float32))
    conv_w = np.asarray(conv_w, dtype=np.float32)
    tc_w = np.asarray(tc_w, dtype=np.float32)
    tc_b = np.asarray(tc_b, dtype=np.float32)
    assert x.shape == (B, S, C), x.shape

    from concourse.bass_utils import run_bass_kernel_spmd

    if "nc" not in _BUILD_CACHE:
        _BUILD_CACHE["nc"] = build_bass()
    nc = _BUILD_CACHE["nc"]

    shared = make_shared_inputs(conv_w, tc_w, tc_b)
    # host-side transposed fp16 views of x (per batch):
    #   xt rows 0:64 = x[t-1, c] (tap0, circular), rows 64:128 = x[t, c]
    #   x2t rows = x[t+1, c] (tap2, circular); device adds tau row on top
    xT = np.transpose(x, (0, 2, 1)).astype(np.float16)  # (B, 64, S)
    xt_full = np.concatenate([np.roll(xT, 1, axis=2), xT], axis=1)  # (B, 128, S)
    x2t_full = np.roll(xT, -1, axis=2)  # (B, 64, S)

    in_maps = []
    for c in range(NCORES):
        m = dict(shared)
        sl = slice(c * BLOC, (c + 1) * BLOC)
        m["xin"] = np.ascontiguousarray(x[sl])
        m["xt"] = np.ascontiguousarray(xt_full[sl])
        m["x2t"] = np.ascontiguousarray(x2t_full[sl])
        in_maps.append(m)

    trace = bool(int(os.environ.get("KERNEL_TRACE", "0")))
    if trace:
        _install_ntff_hook()
    res = run_bass_kernel_spmd(
        nc, in_maps, core_ids=list(range(NCORES)), trace=trace, trace_cores=[0]
    )
    if trace and res.exec_time_ns is not None:
        print(
            f"[kernel] HW exec time: {res.exec_time_ns} ns "
            f"(mean {res.mean_exec_time_ns} ns)"
        )
        kernel.last_exec_time_ns = res.exec_time_ns
        kernel.last_trace = res.instructions_and_trace
    out = np.concatenate([r["out"] for r in res.results], axis=0).astype(np.float32)
    return out


if __name__ == "__main__":
    build_bass()
    print("build ok")
